# revision 10
# baseline (speedup 1.0000x reference)
"""CDGM (graph-construction GNN) fused kernel for Trainium2, 8-way row-sharded.

Math per layer (reference):
    gl   = relu(x @ Wgl + bgl)                      [N, F]
    t_ij = ||gl_i - gl_j||^2  (via sq_i + sq_j - 2 gl_i.gl_j)
    adj  = sigmoid(-(1+temp)*sqrt(relu(t)+eps) * (t>0) + (5+theta))
    x    = (adj @ (x @ Wgnn + bgnn)) / rowsum(adj)   (+relu except last layer)
    out  = softmax(x)

Device strategy (per core r, query rows Q_r = [1024*r, 1024*(r+1))):
  - glTs = sqrt(2)*gl stored transposed [F, N] in fp16; t' = -t computed
    tile-wise in PSUM (f32) as (glTs_j . glTs_q) - sq_j - sq_q via a K=F
    fp16 matmul plus a K=2 rank-2 correction ([-sqT; 1] x [1; -sqT]).
    Since sq is computed from the same quantized glTs, the quadratic form
    stays consistent and the distance cancellation is safe.
  - DVE clamps r = max(-t', 0) -> fp16 slab; ACT Sqrt then ACT Sigmoid run
    in table-batched phases over the slab (in-place) to avoid ACT
    table-set switches.
  - adj tiles (fp16) feed natural-layout matmuls adjT.T @ [h | 1] which
    accumulate y and deg together in PSUM; eviction divides by deg (DVE
    reciprocal + tensor_scalar with fused relu).
  - One AllGather of x1^T (256KB/core fp16) bridges the layers; the final
    division + softmax run on host (y and deg are returned raw in f32).
  - eps is dropped: sqrt(max(t,0)) differs from sqrt(max(t,0)+eps)*mask by
    O(1e-7) relative through the sigmoid.

The harness calls kernel(**inputs) with full inputs; sharding is internal.
"""

import math

import numpy as np

N = 8192
D_IN = 256
F0, F1 = 128, 64
N_CORES = 8
QR = N // N_CORES          # query rows per core
NJ = N // 128              # 64 j-chunks of 128
SLABW = NJ * 512           # slab free width per q-tile (32768)

_CACHE = {}


def _build(temp: float, theta: float):
    import concourse.bacc as bacc
    import concourse.mybir as mybir
    import concourse.tile as tile
    from concourse.tile_rust import add_dep_helper
    from contextlib import ExitStack

    DT = mybir.dt
    AF = mybir.ActivationFunctionType
    ALU = mybir.AluOpType
    F32, FP16 = DT.float32, DT.float16

    sig_scale = -(1.0 + temp)
    sig_bias = 5.0 + theta

    nc = bacc.Bacc(
        "TRN2", target_bir_lowering=False, debug=False, enable_asserts=False,
        num_devices=N_CORES,
    )

    # ---- I/O ----
    xTf_in = nc.dram_tensor("xTf", [D_IN, N], FP16, kind="ExternalInput").ap()
    xTq_in = nc.dram_tensor("xTq", [D_IN, QR], FP16, kind="ExternalInput").ap()
    wglx_in = [
        nc.dram_tensor("wglx0", [D_IN, F0], FP16, kind="ExternalInput").ap(),
        nc.dram_tensor("wglx1", [F0, F1], FP16, kind="ExternalInput").ap(),
    ]
    bglx_in = [
        nc.dram_tensor("bglx0", [F0, 1], F32, kind="ExternalInput").ap(),
        nc.dram_tensor("bglx1", [F1, 1], F32, kind="ExternalInput").ap(),
    ]
    wgna_in = [
        nc.dram_tensor("wgna0", [D_IN + 1, F0 + 1], FP16, kind="ExternalInput").ap(),
        nc.dram_tensor("wgna1", [F0 + 1, F1 + 1], FP16, kind="ExternalInput").ap(),
    ]
    ident_in = nc.dram_tensor("ident", [128, 128], FP16, kind="ExternalInput").ap()
    y_out = nc.dram_tensor("y_out", [QR, F1 + 1], F32, kind="ExternalOutput").ap()

    with tile.TileContext(nc) as tc, ExitStack() as ctx:
        pconst = ctx.enter_context(tc.tile_pool(name="const", bufs=1))
        pouter = ctx.enter_context(tc.tile_pool(name="outer", bufs=1))
        pdram = ctx.enter_context(tc.tile_pool(name="dram", bufs=1, space="DRAM"))
        psA = ctx.enter_context(tc.tile_pool(name="psA", bufs=2, space="PSUM"))
        psB = ctx.enter_context(tc.tile_pool(name="psB", bufs=4, space="PSUM"))

        # ---- constants ----
        ident = pconst.tile([128, 128], FP16, tag="ident")
        nc.sync.dma_start(ident[:], ident_in[:])
        ones16 = pconst.tile([1, 128], FP16, tag="ones16")
        nc.gpsimd.memset(ones16[:], 1.0)
        neghalf = pconst.tile([128, 1], FP16, tag="neghalf")
        nc.gpsimd.memset(neghalf[:], -0.5)
        sgb = pconst.tile([128, 1], F32, tag="sgb")
        nc.gpsimd.memset(sgb[:], sig_bias)
        sgs = pconst.tile([128, 1], F32, tag="sgs")
        nc.gpsimd.memset(sgs[:], sig_scale)

        fins = [D_IN, F0]
        fouts = [F0, F1]
        wgl = []
        bgl = []
        wgna = []
        wgna_ones = []
        for li in range(2):
            fin, fout = fins[li], fouts[li]
            nk = fin // 128
            wk = []
            for k in range(nk):
                t = pconst.tile([128, fout], FP16, tag=f"wgl{li}_{k}")
                nc.sync.dma_start(t[:], wglx_in[li][k * 128:(k + 1) * 128, :])
                wk.append(t)
            wgl.append(wk)
            bt = pconst.tile([fout, 1], F32, tag=f"bgl{li}")
            nc.sync.dma_start(bt[:], bglx_in[li][:])
            bgl.append(bt)
            ak = []
            for k in range(nk):
                t = pconst.tile([128, fout + 1], FP16, tag=f"wgna{li}_{k}")
                nc.sync.dma_start(t[:], wgna_in[li][k * 128:(k + 1) * 128, :])
                ak.append(t)
            wgna.append(ak)
            t = pconst.tile([1, fout + 1], FP16, tag=f"wgnaone{li}")
            nc.sync.dma_start(t[:], wgna_in[li][fin:fin + 1, :])
            wgna_ones.append(t)

        # layer-0 -> layer-1 bridge (x1 transposed, this core's columns)
        x1Tq = pouter.tile([F0, QR], FP16, tag="x1tq")
        agin = pdram.tile([F0, QR], FP16, tag="agin")
        agout = pdram.tile([N_CORES * F0, QR], FP16, tag="agout")

        for li in range(2):
            fin, fout = fins[li], fouts[li]
            nk = fin // 128
            fp = fout + 1
            with ExitStack() as lctx:
                pmain = lctx.enter_context(tc.tile_pool(name=f"main{li}", bufs=1))
                glTs = pmain.tile([fout, N], FP16, tag="glTs")
                glTsq = pmain.tile([fout, QR], FP16, tag="glTsq")
                augL = pmain.tile([2, N], FP16, tag="augL")    # [-sqT ; ones]
                augQ = pmain.tile([2, QR], FP16, tag="augQ")   # [ones ; -sqTq]
                h_nat = pmain.tile([128, NJ * fp], FP16, tag="hnat")
                # Row 1 of a tile is not engine-addressable (start partition
                # must be 0/32/64/96): memset both rows to 1.0, overwrite row 0
                # (augL) directly, and DMA the augQ -sq row in from a staging
                # row on partition 0.
                nc.gpsimd.memset(augL[:], 1.0)
                nc.gpsimd.memset(augQ[:], 1.0)
                sqq_stage = pmain.tile([1, QR], FP16, tag="sqqstage")

                # ======== setup ========
                with ExitStack() as sctx:
                    pxt = sctx.enter_context(tc.tile_pool(name=f"xt{li}", bufs=1))
                    xtf = [pxt.tile([128, N], FP16, tag=f"xtf{k}", name=f"xtf{k}")
                           for k in range(nk)]
                    if li == 0:
                        for k in range(nk):
                            for c in range(4):
                                nc.sync.dma_start(
                                    xtf[k][:, c * 2048:(c + 1) * 2048],
                                    xTf_in[k * 128:(k + 1) * 128, c * 2048:(c + 1) * 2048],
                                )
                        xtq = [pxt.tile([128, QR], FP16, tag=f"xtq{k}", name=f"xtq{k}")
                               for k in range(nk)]
                        for k in range(nk):
                            nc.sync.dma_start(xtq[k][:], xTq_in[k * 128:(k + 1) * 128, :])
                    else:
                        for r in range(N_CORES):
                            nc.sync.dma_start(
                                xtf[0][:, r * QR:(r + 1) * QR],
                                agout[r * F0:(r + 1) * F0, :],
                            )
                        xtq = [x1Tq]

                    # glTs = relu(x @ sqrt2*Wgl + sqrt2*bgl), transposed layout
                    for c in range(N // 512):
                        pg = psA.tile([fout, 512], F32, tag="tmac")
                        for k in range(nk):
                            nc.tensor.matmul(
                                pg[:], wgl[li][k][:], xtf[k][:, c * 512:(c + 1) * 512],
                                start=(k == 0), stop=(k == nk - 1),
                            )
                        nc.scalar.activation(
                            glTs[:, c * 512:(c + 1) * 512], pg[:], AF.Relu,
                            bias=bgl[li][:], scale=1.0,
                        )
                    for c in range(QR // 512):
                        pg = psA.tile([fout, 512], F32, tag="tmac")
                        for k in range(nk):
                            nc.tensor.matmul(
                                pg[:], wgl[li][k][:], xtq[k][:, c * 512:(c + 1) * 512],
                                start=(k == 0), stop=(k == nk - 1),
                            )
                        nc.scalar.activation(
                            glTsq[:, c * 512:(c + 1) * 512], pg[:], AF.Relu,
                            bias=bgl[li][:], scale=1.0,
                        )

                    # -sq rows: augL[0] = -0.5 * colsum(glTs^2)  (glTs = sqrt2*gl)
                    for c in range(N // 512):
                        gl2c = pxt.tile([fout, 512], FP16, tag="gl2c", bufs=2)
                        nc.vector.tensor_mul(
                            gl2c[:], glTs[:, c * 512:(c + 1) * 512],
                            glTs[:, c * 512:(c + 1) * 512],
                        )
                        pq = psB.tile([1, 512], F32, tag="oacc")
                        nc.tensor.matmul(pq[:], neghalf[0:fout, :], gl2c[:],
                                         start=True, stop=True)
                        nc.vector.tensor_copy(augL[0:1, c * 512:(c + 1) * 512], pq[:])
                    for c in range(QR // 512):
                        gl2c = pxt.tile([fout, 512], FP16, tag="gl2c", bufs=2)
                        nc.vector.tensor_mul(
                            gl2c[:], glTsq[:, c * 512:(c + 1) * 512],
                            glTsq[:, c * 512:(c + 1) * 512],
                        )
                        pq = psB.tile([1, 512], F32, tag="oacc")
                        nc.tensor.matmul(pq[:], neghalf[0:fout, :], gl2c[:],
                                         start=True, stop=True)
                        nc.vector.tensor_copy(sqq_stage[0:1, c * 512:(c + 1) * 512], pq[:])
                    nc.sync.dma_start(augQ[1:2, :], sqq_stage[:])

                    # h_nat = [x @ Wgnn + bgnn | 1] fp16, natural layout,
                    # packed per j-chunk at stride fout+1
                    grp = 3
                    for g0 in range(0, NJ, grp):
                        gn = min(grp, NJ - g0)
                        ph = psB.tile([128, grp * fp], F32, tag="oacc")
                        for t in range(gn):
                            n_ = g0 + t
                            sl = ph[:, t * fp:(t + 1) * fp]
                            for k in range(nk):
                                nc.tensor.matmul(
                                    sl, xtf[k][:, n_ * 128:(n_ + 1) * 128], wgna[li][k][:],
                                    start=(k == 0), stop=False,
                                )
                            nc.tensor.matmul(
                                sl, ones16[:], wgna_ones[li][:], start=False, stop=True,
                            )
                        nc.vector.tensor_copy(
                            h_nat[:, g0 * fp:(g0 + gn) * fp], ph[:, 0:gn * fp],
                        )

                # ======== main flash loop ========
                with ExitStack() as mctx:
                    pslab = mctx.enter_context(tc.tile_pool(name=f"slab{li}", bufs=1))
                    slabs = [
                        pslab.tile([128, SLABW], FP16, tag=f"slab{qt}", name=f"slab{qt}")
                        for qt in (0, 1)
                    ]
                    sqrt_insts = []
                    for qt in (0, 1):
                        slab = slabs[qt]
                        for jg in range(NJ // 2):
                            tm = psA.tile([128, 1024], F32, tag="tmac")
                            for h2 in (0, 1):
                                j = jg * 2 + h2
                                sl = tm[:, h2 * 512:(h2 + 1) * 512]
                                nc.tensor.matmul(
                                    sl, glTs[:, j * 128:(j + 1) * 128],
                                    glTsq[:, qt * 512:(qt + 1) * 512],
                                    start=True, stop=False,
                                )
                                nc.tensor.matmul(
                                    sl, augL[:, j * 128:(j + 1) * 128],
                                    augQ[:, qt * 512:(qt + 1) * 512],
                                    start=False, stop=True,
                                )
                            # r = max(t, 0) = max(-t', 0)   (fp16)
                            nc.vector.tensor_scalar(
                                slab[:, jg * 1024:(jg + 1) * 1024], tm[:],
                                -1.0, 0.0, ALU.mult, ALU.max,
                            )
                        for g in range(8):
                            sl = slab[:, g * 4096:(g + 1) * 4096]
                            si = nc.scalar.activation(sl, sl, AF.Sqrt, bias=0.0, scale=1.0)
                            sqrt_insts.append(si)
                    for qt in (0, 1):
                        slab = slabs[qt]
                        for g in range(8):
                            sl = slab[:, g * 4096:(g + 1) * 4096]
                            si = nc.scalar.activation(sl, sl, AF.Sigmoid, bias=sgb[:], scale=sgs[:])
                            # Keep all sigmoids after every sqrt so the ACT
                            # table set switches exactly once per phase.
                            add_dep_helper(si.ins, sqrt_insts[-1].ins, sync=False,
                                           reason="act-table phase batching")
                        oaccs = [psB.tile([128, fp], F32, tag="oacc", name=f"oacc{_s}")
                                 for _s in range(4)]
                        for j in range(NJ):
                            for s in range(4):
                                nc.tensor.matmul(
                                    oaccs[s][:],
                                    slab[:, j * 512 + s * 128:j * 512 + (s + 1) * 128],
                                    h_nat[:, j * fp:(j + 1) * fp],
                                    start=(j == 0), stop=(j == NJ - 1),
                                )
                        for s in range(4):
                            row0 = qt * 512 + s * 128
                            if li == 0:
                                recip = pmain.tile([128, 1], F32, tag="recip", bufs=2)
                                nc.vector.reciprocal(recip[:], oaccs[s][:, fout:fout + 1])
                                x1n = pmain.tile([128, fout], FP16, tag="x1n", bufs=2)
                                nc.vector.tensor_scalar(
                                    x1n[:], oaccs[s][:, 0:fout], recip[:], 0.0,
                                    ALU.mult, ALU.max,
                                )
                                tp = psA.tile([fout, 128], FP16, tag="tmac")
                                nc.tensor.transpose(tp[:], x1n[:], ident[:])
                                nc.vector.tensor_copy(
                                    x1Tq[0:fout, row0:row0 + 128], tp[:],
                                )
                            else:
                                yev = pmain.tile([128, fp], F32, tag="yev", bufs=4)
                                nc.vector.tensor_copy(yev[:], oaccs[s][:])
                                nc.sync.dma_start(
                                    y_out[row0:row0 + 128, :], yev[:],
                                )
            if li == 0:
                nc.gpsimd.dma_start(agin[:], x1Tq[:])
                nc.gpsimd.collective_compute(
                    "AllGather", mybir.AluOpType.bypass,
                    ins=[agin.opt()], outs=[agout.opt()],
                    replica_groups=[list(range(N_CORES))],
                )

    nc.compile()
    return nc


def _prep_in_maps(feat, Wgl0, bgl0, Wgnn0, bgnn0, Wgl1, bgl1, Wgnn1, bgnn1):
    s2 = np.float32(math.sqrt(2.0))
    xT = np.asarray(feat, np.float32).T

    def f32(a):
        return np.asarray(a, np.float32)

    xT16 = np.ascontiguousarray(xT.astype(np.float16))
    wglx0 = np.ascontiguousarray((f32(Wgl0) * s2).astype(np.float16))
    bglx0 = np.ascontiguousarray((f32(bgl0) * s2).reshape(-1, 1))
    wglx1 = np.ascontiguousarray((f32(Wgl1) * s2).astype(np.float16))
    bglx1 = np.ascontiguousarray((f32(bgl1) * s2).reshape(-1, 1))

    def aug(W, b):
        fin, fout = W.shape
        a = np.zeros((fin + 1, fout + 1), np.float16)
        a[:fin, :fout] = f32(W)
        a[fin, :fout] = f32(b)
        a[fin, fout] = 1.0
        return a

    wgna0 = aug(f32(Wgnn0), bgnn0)
    wgna1 = aug(f32(Wgnn1), bgnn1)
    ident = np.eye(128, dtype=np.float16)

    in_maps = []
    for r in range(N_CORES):
        in_maps.append({
            "xTf": xT16,
            "xTq": np.ascontiguousarray(xT16[:, r * QR:(r + 1) * QR]),
            "wglx0": wglx0, "bglx0": bglx0, "wgna0": wgna0,
            "wglx1": wglx1, "bglx1": bglx1, "wgna1": wgna1,
            "ident": ident,
        })
    return in_maps


def _postprocess(results):
    y = np.concatenate(
        [np.asarray(results[r]["y_out"]) for r in range(N_CORES)], axis=0
    )  # [8192, 65]
    x2 = y[:, :F1] / y[:, F1:F1 + 1]
    m = x2.max(axis=-1, keepdims=True)
    e = np.exp(x2 - m)
    return (e / e.sum(axis=-1, keepdims=True)).astype(np.float32)


def kernel(**inputs):
    from concourse.bass_utils import run_bass_kernel_spmd

    feat = np.asarray(inputs["feat_matrix"], np.float32)
    temp = float(np.asarray(inputs["temp"]))
    theta = float(np.asarray(inputs["theta"]))
    key = (round(temp, 9), round(theta, 9))
    if key not in _CACHE:
        _CACHE[key] = _build(temp, theta)
    nc = _CACHE[key]

    in_maps = _prep_in_maps(
        feat, inputs["Wgl0"], inputs["bgl0"], inputs["Wgnn0"], inputs["bgnn0"],
        inputs["Wgl1"], inputs["bgl1"], inputs["Wgnn1"], inputs["bgnn1"],
    )
    res = run_bass_kernel_spmd(nc, in_maps, list(range(N_CORES)))
    return _postprocess(res.results)
